# revision 13
# baseline (speedup 1.0000x reference)
"""Trainium2 Bass kernel for the CNV quantized CNN (nn_CNV_48498770706530), v2.

Data-parallel over 8 NeuronCores: 128 images per core, weights replicated.

All quantized weights/activations are ternary; activations are carried on
device as q+1 in {0,1,2} (fp8e4m3-exact), letting every conv/fc run as fp8
DoubleRow matmuls (two K-blocks per instruction at 0.5 cycles/row).  The
constant offset contribution 1*rowsum(W) is folded into the next layer's
quant bias on the host.  conv0 splits the 8-bit input into hi/lo nibbles
(xq = 16*hi + lo, both fp8-exact) paired as DoubleRow blocks with 16W / W
weights, two images packed block-diagonally.

BatchNorm+aquant collapses to a per-channel integer step function evaluated
with a 3-op chain that is exactly verified on the host:

    ub = bf16(s0*h + d)     ACT/DVE  (cast-on-write rounds; d targets 196)
    t  = min(ub - 195, 2)   DVE      (bf16, 4x perf mode)
    q1 = max(t, 0) -> fp8   Pool/DVE ({0,1,2} exact)

2x2 avgpool+aquant needs no clips (|sp*(s'-4)| < 0.75) and runs as two
fused scalar_tensor_tensor adds + one magic multiply-add, all at DVE 4x.
"""
import os
import sys

for _p in ("/opt/trn_rl_repo", "/root/.axon_site/_ro/trn_rl_repo"):
    if os.path.isdir(_p) and _p not in sys.path:
        sys.path.insert(0, _p)

import contextlib

import numpy as np
import ml_dtypes

import concourse.bass as bass
import concourse.tile as tile
from concourse import bacc
from concourse import mybir
from concourse.ap import AP

F32 = mybir.dt.float32
BF16 = mybir.dt.bfloat16
FP8 = mybir.dt.float8e4
BF = ml_dtypes.bfloat16
NPF8 = ml_dtypes.float8_e4m3
AF = mybir.ActivationFunctionType
ALU = mybir.AluOpType
DR = mybir.MatmulPerfMode.DoubleRow

N_CORES = 8
EPS = np.float32(1e-4)
MAG = 196.0                     # bf16 magic target (ulp 1 in [128,256))
OFFS = 1.0                      # activations stored as q + OFFS
SP0 = np.float32(0.5 / 2.75)    # pool quant scale

V_NM = [1, 1, 1, 1, 2, 2, 4, 4]  # m-tiles per quant stage


# ---------------------------------------------------------------------------
# Host-side exact math
# ---------------------------------------------------------------------------

def _wq(w):
    return np.clip(np.round(np.asarray(w, np.float32)), -1.0, 1.0).astype(np.float32)


def _emul_chain(u32):
    """Emulate device S2/S3 on fp32 u values -> q' in {0,1,2}."""
    ub = u32.astype(BF).astype(np.float32)
    t = np.minimum((ub - np.float32(195.0)).astype(np.float32),
                   np.float32(2.0)).astype(BF).astype(np.float32)
    return np.maximum(t, np.float32(0.0)).astype(NPF8).astype(np.float32)


def _quant_params(bnp, hmax, corr, href_scale=1.0):
    """Per-channel (s0, d) so that on device (h = true + corr, integer):
         chain(bf16(s0*h + d)) == aquant(bn(href)) + 1
    with href = flip*(h - corr)*href_scale.  Exactly verified."""
    g, b, m, v = [np.asarray(x, np.float32) for x in bnp]
    inv = (g / np.sqrt(v + EPS)).astype(np.float32)
    C = inv.shape[0]
    flip = np.where(inv < 0, -1.0, 1.0).astype(np.float32)
    corr = np.asarray(corr, np.float64)
    assert corr.shape == (C,)

    base = np.arange(-hmax, hmax + 1, dtype=np.float64)
    grid = base[None, :] + corr[:, None]                     # device h'
    href = (base[None, :] * flip[:, None] * np.float64(href_scale)
            ).astype(np.float32)
    y = ((href - m[:, None]) * inv[:, None] + b[:, None]).astype(np.float32)
    q = np.clip(np.round(y), -1.0, 1.0)                     # [C, G] in {-1,0,1}
    assert np.all(np.diff(q, axis=1) >= 0), "quant map not monotone"

    s0 = np.empty(C, np.float32)
    d0 = np.empty(C, np.float32)
    for c in range(C):
        qc = q[c]
        gc = grid[c]
        has_hi = qc[-1] > 0.5
        has_lo = qc[0] < -0.5
        if not has_hi and not has_lo:
            s0[c], d0[c] = 0.0, np.float32(MAG + qc[0])
            continue
        Hp = gc[np.argmax(qc > 0.5)] if has_hi else corr[c] + 4.0 * hmax
        Hm = (gc[len(qc) - 1 - np.argmax(qc[::-1] < -0.5)]
              if has_lo else corr[c] - 4.0 * hmax)
        c0 = (Hp + Hm) / 2.0
        W = (Hp - Hm) / 2.0
        for bump in range(12):
            s = np.float64(0.5 / (W - 0.25)) * (1.0 + bump * 2.0 ** -16)
            sc = np.float32(s)
            dc = np.float32(np.float64(MAG) - np.float64(sc) * c0)
            u_fma = (gc * np.float64(sc) + np.float64(dc)).astype(np.float32)
            u_sep = ((gc.astype(np.float32) * sc).astype(np.float32)
                     + dc).astype(np.float32)
            ok = (np.array_equal(_emul_chain(u_fma), qc + 1.0)
                  and np.array_equal(_emul_chain(u_sep), qc + 1.0))
            if ok:
                s0[c], d0[c] = sc, dc
                break
        else:
            raise AssertionError(f"quant params failed for channel {c}")

    # full-grid final verification (vectorized, both interpretations)
    u_fma = (grid * s0[:, None].astype(np.float64)
             + d0[:, None].astype(np.float64)).astype(np.float32)
    u_sep = ((grid.astype(np.float32) * s0[:, None]).astype(np.float32)
             + d0[:, None]).astype(np.float32)
    assert np.array_equal(_emul_chain(u_fma), q + 1.0)
    assert np.array_equal(_emul_chain(u_sep), q + 1.0)
    return flip, np.stack([s0, d0], axis=1)


def _verify_pool():
    """Pool chain: q' in {0,1,2}; s' = sum4 in [0,8];
    ub = bf16(f32(f32(s'-4)*sp) + 196); q_p' = fp8(ub-195) must equal
    aquant(avgpool(q)) + 1.  (s1/s2 adds are exact in bf16.)"""
    sp = SP0
    for s in range(0, 9):
        s2v = np.float32(s - 4)
        u = np.float32(np.float32(s2v * sp) + np.float32(MAG))
        ub = np.float32(BF(u))
        qp = float(NPF8(np.float32(ub - np.float32(195.0))))
        ref = float(np.clip(np.round(np.float32((s - 4) / 4.0)), -1, 1)) + 1.0
        assert qp == ref, (s, qp, ref)


_verify_pool()


def host_prep(inputs, n_cores=N_CORES):
    """Quantize weights, build DR layouts + quant params, im2col conv0 input.

    Returns (const_map, per_core_x0_list)."""
    x = np.asarray(inputs["x"], np.float32)
    batch = x.shape[0]
    wcs = [_wq(inputs[f"wc{i}"]) for i in range(6)]
    bncs = [np.asarray(inputs[f"bnc{i}"], np.float32) for i in range(6)]
    wfs = [_wq(inputs[f"wf{i}"]) for i in range(3)]
    bnf = [np.asarray(inputs["bnf0"], np.float32),
           np.asarray(inputs["bnf1"], np.float32)]
    tn = np.asarray(inputs["tn"], np.float32)

    # --- quant params; corr[L] = OFFS * rowsum(W_dev[L]) (dev = flip*W) ---
    hb = [27 * 128, 576, 576, 1152, 1152, 2304, 256, 512]
    flips, Vs = [], []
    corrs = []
    wdevs = []
    for i in range(6):
        g_, _, _, v_ = bncs[i]
        inv_ = (g_ / np.sqrt(v_ + EPS)).astype(np.float32)
        flip_ = np.where(inv_ < 0, -1.0, 1.0).astype(np.float32)
        wdev_ = (wcs[i] * flip_[:, None, None, None]).astype(np.float32)
        C_ = wcs[i].shape[0]
        corr = (np.zeros(C_) if i == 0
                else OFFS * wdev_.reshape(C_, -1).sum(1))
        f, V = _quant_params(bncs[i], hb[i], corr,
                             href_scale=(1.0 / 128.0 if i == 0 else 1.0))
        assert np.array_equal(f, flip_)
        flips.append(f)
        Vs.append(V)
        corrs.append(corr)
        wdevs.append(wdev_)
    def _fc_params(bnp, hmax, wraw):
        # flip depends only on sign(inv), not on corr
        g, b_, m_, v_ = [np.asarray(t, np.float32) for t in bnp]
        inv = (g / np.sqrt(v_ + EPS)).astype(np.float32)
        flip = np.where(inv < 0, -1.0, 1.0).astype(np.float32)
        wdev = (wraw * flip[:, None]).astype(np.float32)
        f2, V = _quant_params(bnp, hmax, OFFS * wdev.sum(1))
        assert np.array_equal(f2, flip)
        return flip, V, wdev

    f6, V6, wf0dev = _fc_params(bnf[0], hb[6], wfs[0])
    flips.append(f6)
    Vs.append(V6)
    f7, V7, wf1dev = _fc_params(bnf[1], hb[7], wfs[1])
    flips.append(f7)
    Vs.append(V7)

    def vpack(V, dup=False):
        if dup:
            V = np.concatenate([V, V], axis=0)
        nm = V.shape[0] // 128
        return np.ascontiguousarray(
            V.reshape(nm, 128, 2).transpose(1, 2, 0)).astype(np.float32)

    cm = {}
    for i in range(8):
        cm[f"v{i}"] = vpack(Vs[i], dup=(i < 2))

    # --- conv0 weights: block-diag 2-img, DR blocks (16W | W), [54,2,128] ---
    w0 = wdevs[0]                                           # [64,3,3,3]
    w0r = np.ascontiguousarray(
        w0.transpose(2, 3, 1, 0).reshape(27, 64)).astype(np.float32)
    w0t = np.zeros((54, 2, 128), np.float32)
    w0t[0:27, 0, 0:64] = 16.0 * w0r
    w0t[27:54, 0, 64:128] = 16.0 * w0r
    w0t[0:27, 1, 0:64] = w0r
    w0t[27:54, 1, 64:128] = w0r
    cm["w0"] = w0t.astype(NPF8)

    # --- conv1: single-plane X1 [128,900]; DR blocks are (dy,dx) taps at
    # column offsets (dy*30+dx); weights block-diag over (img0|img1):
    # i=0..2: (dy0,dx_i)@dx_i, (dy1,dx_i)@30+dx_i (stride 30)
    # i=3:    (dy2,dx0)@60, (dy2,dx1)@61 (stride 1); i=4: (dy2,dx2)@62, zero
    wt1 = wdevs[1].transpose(1, 2, 3, 0)                    # [64, dy, dx, 64]

    def bd1(w):
        out = np.zeros((128, 128), np.float32)
        out[0:64, 0:64] = w
        out[64:128, 64:128] = w
        return out

    w1n = np.zeros((128, 5, 2, 128), np.float32)
    for dx in range(3):
        w1n[:, dx, 0] = bd1(wt1[:, 0, dx])
        w1n[:, dx, 1] = bd1(wt1[:, 1, dx])
    w1n[:, 3, 0] = bd1(wt1[:, 2, 0])
    w1n[:, 3, 1] = bd1(wt1[:, 2, 1])
    w1n[:, 4, 0] = bd1(wt1[:, 2, 2])
    cm["w1n"] = np.ascontiguousarray(w1n).astype(NPF8)

    # conv2: K=128 zero-padded per-image variants from single [128,196] X2:
    # i=0..2: blocks (dy0,dx_i)@dx_i, (dy1,dx_i)@14+dx_i (stride 14)
    # i=3:    (dy2,dx0)@28, (dy2,dx1)@29; i=4: (dy2,dx2)@30, zero
    w2t = wdevs[2].transpose(1, 2, 3, 0)                    # [64, dy, dx, 128]
    w2n = np.zeros((64, 5, 2, 128), np.float32)
    for dx in range(3):
        w2n[:, dx, 0] = w2t[:, 0, dx]
        w2n[:, dx, 1] = w2t[:, 1, dx]
    w2n[:, 3, 0] = w2t[:, 2, 0]
    w2n[:, 3, 1] = w2t[:, 2, 1]
    w2n[:, 4, 0] = w2t[:, 2, 2]
    w2p = np.zeros((2, 128, 5, 2, 128), np.float32)
    w2p[0, 0:64] = w2n
    w2p[1, 64:128] = w2n
    cm["w2n"] = np.ascontiguousarray(
        w2p.transpose(1, 0, 2, 3, 4)).astype(NPF8)          # [128,2,5,2,128]

    # --- conv3 DR weights: [128, 5, 2, 128]; offset pairs of r=dy*3+dx:
    #     (0,1),(2,3),(4,5),(6,7),(8,zero) ---
    w3 = wdevs[3].transpose(1, 2, 3, 0).reshape(128, 9, 128)  # [C, r, O]
    w3t = np.zeros((128, 5, 2, 128), np.float32)
    for i in range(4):
        w3t[:, i, 0] = w3[:, 2 * i]
        w3t[:, i, 1] = w3[:, 2 * i + 1]
    w3t[:, 4, 0] = w3[:, 8]
    cm["w3"] = np.ascontiguousarray(w3t).astype(NPF8)

    # --- conv4: [128, 2(mh), 5, 2, 128] same pairing ---
    w4 = wdevs[4].reshape(2, 128, 128, 3, 3).transpose(
        2, 0, 3, 4, 1).reshape(128, 2, 9, 128)             # [C, mh, r, O]
    cm["w4"] = np.ascontiguousarray(w4).astype(NPF8)

    # --- conv5: [128, 2(mh), 9, 2(ch), 128] DR over ch halves ---
    w5 = wdevs[5].reshape(2, 128, 2, 128, 3, 3).transpose(
        3, 0, 2, 4, 5, 1).reshape(128, 2, 2, 9, 128)       # [C,mh,ch,r,O]
    cm["w5"] = np.ascontiguousarray(
        w5.transpose(0, 1, 3, 2, 4)).astype(NPF8)          # [C,mh,r,ch,O]

    # --- fc weights ---
    wf0t = wf0dev.reshape(4, 128, 2, 128).transpose(3, 2, 0, 1)  # [C,ch,mt,O]
    cm["wf0"] = np.ascontiguousarray(wf0t).astype(NPF8)
    wf1t = wf1dev.reshape(4, 128, 2, 2, 128).transpose(4, 2, 3, 0, 1)
    # [C, chpair(2), blk(2), mt(4), O] : DR blocks over blk
    cm["wf1"] = np.ascontiguousarray(wf1t).astype(NPF8)
    cm["wf2"] = np.ascontiguousarray(
        wfs[2].reshape(10, 4, 128).transpose(2, 1, 0)).astype(NPF8)

    # --- final TensorNorm with fc2 offset correction (per-channel bias) ---
    w_t, b_t, m_t, v_t = [np.float32(t) for t in tn]
    s_t = np.float32(np.sqrt(np.float32(v_t + EPS)))
    At = np.float32(w_t / s_t)
    corr2 = OFFS * wfs[2].sum(1)                           # [10]
    Bt = (np.float32(b_t - m_t * At)
          - (At * corr2).astype(np.float32)).astype(np.float32)
    tnab = np.zeros((10, 2), np.float32)
    tnab[:, 0] = At
    tnab[:, 1] = Bt
    cm["tnab"] = tnab

    # --- input: inquant*128, hi/lo split, im2col, block-diag pairs ---
    xq = np.clip(np.round(x * np.float32(128.0)), -128.0, 127.0)
    hi = np.round(xq / 16.0)
    lo = xq - 16.0 * hi
    assert np.abs(hi).max() <= 8 and np.abs(lo).max() <= 8
    npairs_total = batch // 2
    x0 = np.zeros((54, 2, npairs_total, 900), np.float32)
    for dy in range(3):
        for dx in range(3):
            r0 = (dy * 3 + dx) * 3
            for blk, src in ((0, hi), (1, lo)):
                pat = src[:, :, dy:dy + 30, dx:dx + 30].reshape(batch, 3, 900)
                x0[r0:r0 + 3, blk, :, :] = pat[0::2].transpose(1, 0, 2)
                x0[27 + r0:27 + r0 + 3, blk, :, :] = pat[1::2].transpose(1, 0, 2)
    x0 = np.ascontiguousarray(
        x0.reshape(54, 2, n_cores, npairs_total // n_cores, 900)
        .transpose(2, 0, 1, 3, 4)).astype(NPF8)
    per_core = [x0[c] for c in range(n_cores)]
    return cm, per_core


# ---------------------------------------------------------------------------
# Device program
# ---------------------------------------------------------------------------

def _blk(apbase, extra, offset_elems):
    """Custom AP: partition dim of `apbase` + extra [stride,count] dims."""
    return AP(tensor=apbase.tensor,
              ap=[list(apbase.ap[0])] + [list(e) for e in extra],
              offset=apbase.offset + offset_elems)


def build_bass(PC, debug_taps=False):
    """Per-core Bass program for PC images (PC % 32 == 0)."""
    assert PC % 32 == 0
    nc = bacc.Bacc("TRN2", target_bir_lowering=False, debug=False)
    npairs = PC // 2
    G4 = 16

    d = {}
    d["x0"] = nc.dram_tensor("x0", [54, 2, npairs, 900], FP8,
                             kind="ExternalInput")
    d["w0"] = nc.dram_tensor("w0", [54, 2, 128], FP8, kind="ExternalInput")
    d["w1n"] = nc.dram_tensor("w1n", [128, 5, 2, 128], FP8,
                              kind="ExternalInput")
    d["w2n"] = nc.dram_tensor("w2n", [128, 2, 5, 2, 128], FP8,
                              kind="ExternalInput")
    d["w3"] = nc.dram_tensor("w3", [128, 5, 2, 128], FP8, kind="ExternalInput")
    d["w4"] = nc.dram_tensor("w4", [128, 2, 9, 128], FP8,
                             kind="ExternalInput")
    d["w5"] = nc.dram_tensor("w5", [128, 2, 9, 2, 128], FP8,
                             kind="ExternalInput")
    d["wf0"] = nc.dram_tensor("wf0", [128, 2, 4, 128], FP8,
                              kind="ExternalInput")
    d["wf1"] = nc.dram_tensor("wf1", [128, 2, 2, 4, 128], FP8,
                              kind="ExternalInput")
    d["wf2"] = nc.dram_tensor("wf2", [128, 4, 10], FP8,
                              kind="ExternalInput")
    for i, nm in enumerate(V_NM):
        d[f"v{i}"] = nc.dram_tensor(f"v{i}", [128, 2, nm], F32,
                                    kind="ExternalInput")
    d["tnab"] = nc.dram_tensor("tnab", [10, 2], F32, kind="ExternalInput")
    out_d = nc.dram_tensor("out", [PC, 10], F32, kind="ExternalOutput")
    taps = {}
    if debug_taps:
        for nm, shape in (("tX1", [128, 900]), ("tX2", [128, 196]),
                          ("tX3", [128, 2, 144]), ("tq1", [128, 784]),
                          ("tA4", [128, 16, 25]), ("tA5", [128, PC, 2, 9]),
                          ("tA6", [128, PC, 2]), ("tA7", [128, PC, 4]),
                          ("tA8", [128, PC, 4]),
                          ("tps1", [128, 2, 420])):
            taps[nm] = nc.dram_tensor(nm, shape, F32, kind="ExternalOutput")

    with contextlib.ExitStack() as ctx:
        tc = ctx.enter_context(tile.TileContext(nc))
        wp = ctx.enter_context(tc.tile_pool(name="weights", bufs=1))
        xp = ctx.enter_context(tc.tile_pool(name="xp", bufs=6))
        io = ctx.enter_context(tc.tile_pool(name="io", bufs=4))
        wk = ctx.enter_context(tc.tile_pool(name="work", bufs=4))
        st = ctx.enter_context(tc.tile_pool(name="stage", bufs=1))
        pp0 = ctx.enter_context(tc.tile_pool(name="pp0", bufs=1, space="PSUM"))
        pp1 = ctx.enter_context(tc.tile_pool(name="pp1", bufs=1, space="PSUM"))
        pp23 = ctx.enter_context(tc.tile_pool(name="pp23", bufs=2,
                                              space="PSUM"))
        pp4 = ctx.enter_context(tc.tile_pool(name="pp4", bufs=2, space="PSUM"))

        # prefetch first pairs' inputs ahead of the weight bulk
        x0pre = []
        for p_ in range(min(6, npairs)):
            t_ = xp.tile([54, 2, 900], FP8, tag="x0s")
            nc.sync.dma_start(t_[:], d["x0"][:, :, p_, :])
            x0pre.append(t_)

        # warmup ACT (activation-table load off the critical path)
        warm = wp.tile([128, 8], F32, tag="actwarm")
        nc.vector.memset(warm[:], 0.0)
        nc.scalar.activation(warm[:], warm[:], AF.Identity)

        def wload(name, shape, part=None, eng=None):
            t = wp.tile(shape, FP8, tag=name)
            e = eng or nc.scalar
            if part is None:
                e.dma_start(t[:], d[name][:])
            else:
                e.dma_start(t[part[0]:part[1]], d[name][:])
            return t

        w0s = wload("w0", [54, 2, 128], eng=nc.sync)
        w1ns = wload("w1n", [128, 5, 2, 128], eng=nc.sync)
        w2ns = wload("w2n", [128, 2, 5, 2, 128], eng=nc.sync)
        w3s = wload("w3", [128, 5, 2, 128], eng=nc.sync)
        w4s = wload("w4", [128, 2, 9, 128], eng=nc.scalar)
        w5s = wload("w5", [128, 2, 9, 2, 128], eng=nc.scalar)
        wf0s = wload("wf0", [128, 2, 4, 128], eng=nc.scalar)
        wf1s = wload("wf1", [128, 2, 2, 4, 128], eng=nc.scalar)
        wf2s = wload("wf2", [128, 4, 10], eng=nc.scalar)
        vs = []
        for i, nm in enumerate(V_NM):
            t = wp.tile([128, 2, nm], F32, tag=f"v{i}")
            (nc.sync if i < 4 else nc.scalar).dma_start(t[:], d[f"v{i}"][:])
            vs.append(t)
        tns = wp.tile([10, 2], F32, tag="tnab")
        nc.scalar.dma_start(tns[:], d["tnab"][:])

        # persistent staging
        A5 = st.tile([128, PC, 2, 9], FP8)    # conv5 input acts
        A6 = st.tile([128, PC, 2], FP8)       # fc0 input
        A7 = st.tile([128, PC, 4], FP8)       # fc1 input
        A8 = st.tile([128, PC, 4], FP8)       # fc2 input

        X1sh = 900              # X1 plane stride (elems)

        ps3 = None
        A4 = None

        HP = PC // 2
        a5b = A5[:]
        a6b = A6[:]
        a7b = A7[:]
        a8b = A8[:]

        def tail_half(h):
            """conv5 + fc0/fc1/fc2 + TensorNorm for images h*HP..(h+1)*HP."""
            I0 = h * HP
            for mh in range(2):
                ps5 = pp4.tile([128, 512], F32, tag="ps4")
                for kk in range(9):
                    rhs = AP(tensor=a5b.tensor,
                             ap=[list(a5b.ap[0]), [9, 2], [18, HP]],
                             offset=a5b.offset + 18 * I0 + kk)
                    nc.tensor.matmul(ps5[:, 0:HP], w5s[:, mh, kk], rhs,
                                     start=(kk == 0), stop=(kk == 8),
                                     perf_mode=DR)
                ub5 = wk.tile([128, HP], BF16, tag="ub5")
                nc.vector.tensor_scalar(ub5[:], ps5[:, 0:HP],
                                        vs[5][:, 0, mh:mh + 1],
                                        vs[5][:, 1, mh:mh + 1],
                                        ALU.mult, ALU.add)
                t5 = wk.tile([128, HP], BF16, tag="t5")
                nc.vector.tensor_scalar(t5[:], ub5[:], 195.0, 2.0,
                                        ALU.subtract, ALU.min)
                nc.gpsimd.tensor_scalar(A6[:, I0:I0 + HP, mh], t5[:],
                                        0.0, None, ALU.max)
            for mt in range(4):
                ps = pp23.tile([128, 512], F32, tag="ps23")
                rhs = AP(tensor=a6b.tensor,
                         ap=[list(a6b.ap[0]), [1, 2], [2, HP]],
                         offset=a6b.offset + 2 * I0)
                nc.tensor.matmul(ps[:, 0:HP], wf0s[:, :, mt], rhs,
                                 start=True, stop=True, perf_mode=DR)
                ub = wk.tile([128, HP], BF16, tag="ubf")
                nc.vector.tensor_scalar(ub[:], ps[:, 0:HP],
                                        vs[6][:, 0, mt:mt + 1],
                                        vs[6][:, 1, mt:mt + 1],
                                        ALU.mult, ALU.add)
                t = wk.tile([128, HP], BF16, tag="tf")
                nc.vector.tensor_scalar(t[:], ub[:], 195.0, 2.0,
                                        ALU.subtract, ALU.min)
                nc.gpsimd.tensor_scalar(A7[:, I0:I0 + HP, mt], t[:],
                                        0.0, None, ALU.max)
            for mt in range(4):
                ps = pp23.tile([128, 512], F32, tag="ps23")
                for cp in range(2):
                    rhs = AP(tensor=a7b.tensor,
                             ap=[list(a7b.ap[0]), [1, 2], [4, HP]],
                             offset=a7b.offset + 4 * I0 + 2 * cp)
                    nc.tensor.matmul(ps[:, 0:HP], wf1s[:, cp, :, mt], rhs,
                                     start=(cp == 0), stop=(cp == 1),
                                     perf_mode=DR)
                ub = wk.tile([128, HP], BF16, tag="ubf")
                nc.vector.tensor_scalar(ub[:], ps[:, 0:HP],
                                        vs[7][:, 0, mt:mt + 1],
                                        vs[7][:, 1, mt:mt + 1],
                                        ALU.mult, ALU.add)
                t = wk.tile([128, HP], BF16, tag="tf")
                nc.vector.tensor_scalar(t[:], ub[:], 195.0, 2.0,
                                        ALU.subtract, ALU.min)
                nc.gpsimd.tensor_scalar(A8[:, I0:I0 + HP, mt], t[:],
                                        0.0, None, ALU.max)
            psf = pp4.tile([128, 512], F32, tag="ps4")
            for ch in range(4):
                rhs = AP(tensor=a8b.tensor,
                         ap=[list(a8b.ap[0]), [4, HP]],
                         offset=a8b.offset + 4 * I0 + ch)
                nc.tensor.matmul(psf[0:10, 0:HP], wf2s[:, ch], rhs,
                                 start=(ch == 0), stop=(ch == 3))
            ofc = wk.tile([10, HP], F32, tag="ofc")
            nc.scalar.activation(ofc[:], psf[0:10, 0:HP], AF.Identity,
                                 bias=tns[0:10, 1:2], scale=tns[0:10, 0:1])
            nc.sync.dma_start(out_d[I0:I0 + HP, :].rearrange("i c -> c i"),
                              ofc[:])


        for p in range(npairs):
            # ---------------- conv0 ------------------------------------
            if p < len(x0pre):
                x0s = x0pre[p]
            else:
                x0s = xp.tile([54, 2, 900], FP8, tag="x0s")
                nc.sync.dma_start(x0s[:], d["x0"][:, :, p, :])
            ps0 = pp0.tile([128, 1024], F32, tag="ps0")
            xb = x0s[:]
            for t in range(2):
                rhs = _blk(xb, [[900, 2], [1, 450]], 450 * t)
                nc.tensor.matmul(ps0[:, 512 * t:512 * t + 450], w0s[:], rhs,
                                 start=True, stop=True, perf_mode=DR)
            # quant conv0: S1 on ACT (2-bank strided read), S2 DVE, S3 Pool x2
            ub0 = wk.tile([128, 2, 450], BF16, tag="ub0")
            ps0v = ps0[:].rearrange("p (b c) -> p b c", b=2)
            nc.scalar.activation(ub0[:], ps0v[:, :, 0:450], AF.Identity,
                                 bias=vs[0][:, 1, 0:1], scale=vs[0][:, 0, 0:1])
            t0 = wk.tile([128, 2, 450], BF16, tag="t0")
            nc.vector.tensor_scalar(t0[:], ub0[:], 195.0, 2.0,
                                    ALU.subtract, ALU.min)
            X1 = io.tile([128, 900], FP8, tag="X1")
            nc.gpsimd.tensor_scalar(X1[:], t0[:].rearrange("p b c -> p (b c)"),
                                    0.0, None, ALU.max)
            x1b = X1[:]

            if debug_taps and p == 0:
                _tt = wk.tile([128, 900], F32, tag="dbg1")
                nc.vector.tensor_copy(_tt[:], X1[:])
                nc.sync.dma_start(taps["tX1"][:], _tt[:])
            # ---------------- conv1 (N=418 contiguous, 2 junk cols/row) --
            ps1 = pp1.tile([128, 1024], F32, tag="ps1")
            N1 = 418
            c1off = [(0, 30), (1, 31), (2, 32), (60, 61), (62, 62)]
            for t in range(2):
                co = 30 * 14 * t
                for i, (o_a, o_b) in enumerate(c1off):
                    rhs = _blk(x1b, [[o_b - o_a, 2], [1, N1]], co + o_a)
                    nc.tensor.matmul(ps1[:, 512 * t:512 * t + N1],
                                     w1ns[:, i], rhs,
                                     start=(i == 0), stop=(i == 4),
                                     perf_mode=DR)
            if debug_taps and p == 0:
                _tp = wk.tile([128, 2, 420], F32, tag="dbgp")
                _pv = ps1[:].rearrange("p (b c) -> p b c", b=2)
                nc.vector.tensor_copy(_tp[:, :, 0:418], _pv[:, :, 0:418])
                nc.vector.memset(_tp[:, :, 418:420], 0.0)
                nc.sync.dma_start(taps["tps1"][:], _tp[:])
            # quant conv1 -> q1 bf16 {0,1,2} (parts: img0 ch @0:64)
            ub1 = wk.tile([128, 2, 14, 28], BF16, tag="ub1")
            ps1v = ps1[:].rearrange("p (b c) -> p b c", b=2)
            ps1w = ps1v[:, :, 0:420].rearrange("p b (y x) -> p b y x", x=30)
            nc.scalar.activation(ub1[:], ps1w[:, :, :, 0:28], AF.Identity,
                                 bias=vs[1][:, 1, 0:1], scale=vs[1][:, 0, 0:1])
            t1 = wk.tile([128, 28, 28], BF16, tag="t1")
            nc.vector.tensor_scalar(t1[:].rearrange("p y x -> p (y x)"),
                                    ub1[:].rearrange("p b y x -> p (b y x)"),
                                    195.0, 2.0, ALU.subtract, ALU.min)
            q1o = wk.tile([128, 28, 14], BF16, tag="q1o")
            nc.gpsimd.tensor_scalar(q1o[:].rearrange("p y x -> p (y x)"),
                                    t1[:, :, 1:28:2].rearrange(
                                        "p y x -> p (y x)"),
                                    0.0, None, ALU.max)
            # pool1: fused STT adds (clip even half inline) + magic; all DVE
            s1 = wk.tile([128, 28, 14], BF16, tag="s1p1")
            nc.vector.scalar_tensor_tensor(s1[:], t1[:, :, 0:28:2], 0.0,
                                           q1o[:], ALU.max, ALU.add)
            ubp = wk.tile([128, 196], BF16, tag="ubp1")
            s2v = wk.tile([128, 14, 14], BF16, tag="s2p1")
            nc.vector.scalar_tensor_tensor(s2v[:], s1[:, 0:28:2, :], -4.0,
                                           s1[:, 1:28:2, :], ALU.add, ALU.add)
            nc.vector.tensor_scalar(ubp[:],
                                    s2v[:].rearrange("p a b -> p (a b)"),
                                    float(SP0), MAG, ALU.mult, ALU.add)
            X2 = io.tile([128, 196], FP8, tag="X2")
            nc.gpsimd.tensor_scalar(X2[:], ubp[:], 195.0, None, ALU.subtract)
            x2b = X2[:]
            p2str = x2b.ap[0][0]

            if debug_taps and p == 0:
                _t2 = wk.tile([128, 196], F32, tag="dbg2")
                nc.vector.tensor_copy(_t2[:], X2[:])
                nc.sync.dma_start(taps["tX2"][:], _t2[:])
                _tq = wk.tile([128, 28, 28], F32, tag="dbgq")
                nc.vector.tensor_scalar(
                    _tq[:].rearrange("p y x -> p (y x)"),
                    t1[:].rearrange("p y x -> p (y x)"), 0.0, None, ALU.max)
                nc.sync.dma_start(
                    taps["tq1"][:],
                    _tq[:].rearrange("p y x -> p (y x)"))
            # ---------------- conv2 (K=64 DR per img, N=166) ------------
            ps2 = pp23.tile([128, 512], F32, tag="ps23")
            N2 = 166
            c2off = [(0, 14), (1, 15), (2, 16), (28, 29), (30, 30)]
            for v in range(2):
                no = 168 * v
                for i, (o_a, o_b) in enumerate(c2off):
                    rhs = _blk(x2b, [[o_b - o_a, 2], [1, N2]], o_a)
                    nc.tensor.matmul(ps2[:, no:no + N2],
                                     w2ns[:, v, i], rhs,
                                     start=(i == 0), stop=(i == 4),
                                     perf_mode=DR)
            # quant conv2: S1 on ACT (strided psum read skips junk cols)
            ps2w = ps2[:, 0:336].rearrange("p (i y x) -> p i y x", i=2, x=14)
            ub2 = wk.tile([128, 2, 12, 12], BF16, tag="ub2")
            nc.scalar.activation(ub2[:], ps2w[:, :, 0:12, 0:12], AF.Identity,
                                 bias=vs[2][:, 1, 0:1], scale=vs[2][:, 0, 0:1])
            t2 = wk.tile([128, 288], BF16, tag="t2")
            nc.vector.tensor_scalar(t2[:],
                                    ub2[:].rearrange("p i y x -> p (i y x)"),
                                    195.0, 2.0, ALU.subtract, ALU.min)
            X3 = io.tile([128, 2, 144], FP8, tag="X3")
            nc.gpsimd.tensor_scalar(X3[:].rearrange("p i c -> p (i c)"),
                                    t2[:], 0.0, None, ALU.max)

            if debug_taps and p == 0:
                _t3 = wk.tile([128, 2, 144], F32, tag="dbg3")
                nc.vector.tensor_copy(_t3[:], X3[:])
                nc.sync.dma_start(taps["tX3"][:], _t3[:])
            # ---------------- conv3 (batched 2 pairs, N=118/img) --------
            if p % 2 == 0:
                ps3 = pp23.tile([128, 512], F32, tag="ps23")
            x3b = X3[:]
            N3 = 118
            for ii in range(2):
                no3 = 120 * (2 * (p % 2) + ii)
                for i in range(5):
                    off = [(0, 1), (2, 3), (4, 5), (6, 7), (8, 8)][i]
                    o_a = (off[0] // 3) * 12 + off[0] % 3
                    o_b = (off[1] // 3) * 12 + off[1] % 3
                    rhs = AP(tensor=x3b.tensor,
                             ap=[list(x3b.ap[0]), [o_b - o_a, 2], [1, N3]],
                             offset=x3b.offset + 144 * ii + o_a)
                    nc.tensor.matmul(ps3[:, no3:no3 + N3], w3s[:, i], rhs,
                                     start=(i == 0), stop=(i == 4),
                                     perf_mode=DR)
            if p % 2 == 1:
                # quant conv3 -> q3 bf16 (strided psum read skips junk)
                ps3w = ps3[:, 0:480].rearrange("p (g y x) -> p g y x",
                                               g=4, x=12)
                ub3 = wk.tile([128, 4, 10, 10], BF16, tag="ub3")
                nc.vector.tensor_scalar(ub3[:], ps3w[:, :, 0:10, 0:10],
                                        vs[3][:, 0, 0:1], vs[3][:, 1, 0:1],
                                        ALU.mult, ALU.add)
                t3 = wk.tile([128, 4, 10, 10], BF16, tag="t3")
                nc.vector.tensor_scalar(
                    t3[:].rearrange("p g y x -> p (g y x)"),
                    ub3[:].rearrange("p g y x -> p (g y x)"),
                    195.0, 2.0, ALU.subtract, ALU.min)
                q3o = wk.tile([128, 4, 10, 5], BF16, tag="q3o")
                nc.gpsimd.tensor_scalar(
                    q3o[:].rearrange("p g y x -> p (g y x)"),
                    t3[:, :, :, 1:10:2].rearrange("p g y x -> p (g y x)"),
                    0.0, None, ALU.max)
                s13 = wk.tile([128, 4, 10, 5], BF16, tag="s1p3")
                nc.vector.scalar_tensor_tensor(s13[:], t3[:, :, :, 0:10:2],
                                               0.0, q3o[:], ALU.max, ALU.add)
                s23 = wk.tile([128, 4, 5, 5], BF16, tag="s2p3")
                nc.vector.scalar_tensor_tensor(s23[:], s13[:, :, 0:10:2, :],
                                               -4.0, s13[:, :, 1:10:2, :],
                                               ALU.add, ALU.add)
                ubp3 = wk.tile([128, 100], BF16, tag="ubp3")
                nc.vector.tensor_scalar(
                    ubp3[:], s23[:].rearrange("p i y x -> p (i y x)"),
                    float(SP0), MAG, ALU.mult, ALU.add)
                if (p // 2) % 4 == 0:
                    A4 = io.tile([128, G4, 25], FP8, tag="A4")
                slot = ((p // 2) % 4) * 4
                nc.gpsimd.tensor_scalar(
                    A4[:, slot:slot + 4, :],
                    ubp3[:].rearrange("p (g c) -> p g c", g=4),
                    195.0, None, ALU.subtract)

                # ------------ conv4 (every 16 images) ------------------
                if slot + 4 == G4:
                    g0 = (p // 8) * G4
                    a4b = A4[:]
                    A4v = A4[:].rearrange("p g (y x) -> p g y x", x=5)
                    for mh in range(2):
                        ps4 = pp4.tile([128, 512], F32, tag="ps4")
                        for dy in range(3):
                            for dx in range(3):
                                nc.tensor.matmul(
                                    ps4[:, 0:G4 * 9], w4s[:, mh, dy * 3 + dx],
                                    A4v[:, :, dy:dy + 3, dx:dx + 3],
                                    start=(dy == 0 and dx == 0),
                                    stop=(dy == 2 and dx == 2))
                        # quant conv4: S1 ACT, S2 DVE, S3 Pool
                        ub4 = wk.tile([128, G4 * 9], BF16, tag="ub4")
                        nc.vector.tensor_scalar(ub4[:], ps4[:, 0:G4 * 9],
                                                vs[4][:, 0, mh:mh + 1],
                                                vs[4][:, 1, mh:mh + 1],
                                                ALU.mult, ALU.add)
                        t4 = wk.tile([128, G4 * 9], BF16, tag="t4")
                        nc.vector.tensor_scalar(t4[:], ub4[:], 195.0, 2.0,
                                                ALU.subtract, ALU.min)
                        nc.gpsimd.tensor_scalar(
                            A5[:, g0:g0 + G4, mh, :],
                            t4[:].rearrange("p (g c) -> p g c", g=G4),
                            0.0, None, ALU.max)
                    if p == npairs // 2 - 1:
                        tail_half(0)

        if debug_taps:
            _t4 = wk.tile([128, 16, 25], F32, tag="dbg4")
            nc.vector.tensor_copy(_t4[:], A4[:])
            nc.sync.dma_start(taps["tA4"][:], _t4[:])
            _t5 = wk.tile([128, PC, 2, 9], F32, tag="dbg5")
            nc.vector.tensor_copy(_t5[:], A5[:])
            nc.sync.dma_start(taps["tA5"][:], _t5[:])
        tail_half(1)
        if debug_taps:
            _t6 = wk.tile([128, PC, 2], F32, tag="dbg6")
            nc.vector.tensor_copy(_t6[:], A6[:])
            nc.sync.dma_start(taps["tA6"][:], _t6[:])
            _t7 = wk.tile([128, PC, 4], F32, tag="dbg7")
            nc.vector.tensor_copy(_t7[:], A7[:])
            nc.sync.dma_start(taps["tA7"][:], _t7[:])
            _t8 = wk.tile([128, PC, 4], F32, tag="dbg8")
            nc.vector.tensor_copy(_t8[:], A8[:])
            nc.sync.dma_start(taps["tA8"][:], _t8[:])

    nc.compile()
    return nc


# ---------------------------------------------------------------------------
# Entry point
# ---------------------------------------------------------------------------

def kernel(**inputs) -> np.ndarray:
    from concourse.bass_utils import run_bass_kernel_spmd

    x = np.asarray(inputs["x"])
    batch = x.shape[0]
    pc = batch // N_CORES
    cm, per_core_x0 = host_prep(inputs, N_CORES)
    nc = build_bass(pc)
    in_maps = []
    for c in range(N_CORES):
        m = dict(cm)
        m["x0"] = per_core_x0[c]
        in_maps.append(m)
    res = run_bass_kernel_spmd(nc, in_maps, core_ids=list(range(N_CORES)))
    out = np.concatenate([res.results[c]["out"] for c in range(N_CORES)],
                         axis=0)
    return out.astype(np.float32)
